# revision 24
# baseline (speedup 1.0000x reference)
"""Trainium2 Bass kernel for LowDimProjectedAttention.

Model (reference):
  Q = x @ Wq.T + bq ; K,V likewise  (d_model=2048 -> r=512)
  16 heads of d_k=32, softmax(QK^T/sqrt(32)) @ V, then out_proj r->d_model.
  B=2, S=2048. mask is all-ones (verified by spec fill), dropout p=0.

Sharding (8 cores): core c handles batch b=c//4 and heads 4j..4j+4 where
j=c%4 (i.e. 128 of the 512 r-channels, column-parallel QKV). Attention is
fully local per core. A 4-way AllGather inside each batch group rebuilds
attn_out^T, after which each core computes a 512-wide slice of the output
d_model dimension (column-parallel out_proj, bias folded per-partition).

Schedule: the ACT engine's exp of the S x S x 4-head scores (~16.8M
elements/core, 1 elem/lane/cycle @1.2GHz) is the per-core floor (~110us),
so everything else is arranged to hide under it: QKV projection runs as a
dense PE prologue, then attention is emitted so the exp stream never
starves — scores ping-pong between a 3-bank and a 2-bank PSUM pool while
AV + denominator matmuls interleave per k-chunk. out_proj and the
AllGather staging loads are emitted at deeply lowered scheduler priority
so they only fill engine-idle slots: the collectives' completion time is
wildly variable (shared cc-stream head-of-line blocking was measured at
up to 140us) and anything ordered ahead of pending attention work turns
that variability into a full-pipeline stall.

Every matmul operand is bf16 (fp32 lives only in PSUM accumulation, the
softmax reciprocal, biases, and the final output): f32r matmuls lower to
fp32_mode=HIGH which streams ~3x slower than bf16 and double-pumps
LDWEIGHTS, and bf16 halves the x DMA stream (8 MB/core) besides. x is
loaded in [128, 1024] tiles (2 KB DMA lines) feeding two token tiles'
PSUM accumulators per pass.

Layouts: all activations live transposed on-chip ([feature, token]); the
host pre-transposes x and the weights so no on-device transpose of x is
ever needed. Scores are computed as S^T[k,q] tiles so softmax's divisor
is accumulated with ones-matmuls and the AV product needs V in natural
[k,d] layout, obtained with 16 PE transposes of V^T. The softmax scale
1/sqrt(32) is folded into Wq/bq on the host.
"""

import math

import numpy as np

B = 2
S = 2048
D_MODEL = 2048
R = 512
N_HEADS = 16
D_K = 32
N_CORES = 8
GROUP = 4          # cores per batch group
RLOC = 128         # r-channels per core (4 heads x 32)
NH = 4             # heads per core
TQ = 512           # q tile size
NQT = S // TQ      # 4 q tiles
NKT = S // 128     # 16 k chunks
NDM = D_MODEL // 128  # 16 d_model chunks
LOW_PRIO = -1_000_000  # scheduler priority offset for gap-filler work

_CACHE = {}
TRACE = False
LAST_RESULT = None


def _build():
    import concourse.mybir as mybir
    import concourse.tile as tile
    from concourse import bacc
    from concourse.masks import make_identity

    F32 = mybir.dt.float32
    BF16 = mybir.dt.bfloat16

    # Bacc (not plain Bass): its finalize() runs move_matmul_waits_to_
    # ldweights / generate_event_semaphores etc., without which walrus
    # rejects multi-wait instructions ("Too many sync wait commands").
    nc = bacc.Bacc("TRN2", target_bir_lowering=False, num_devices=N_CORES)

    xT = nc.dram_tensor("xT", [D_MODEL, S], BF16, kind="ExternalInput")
    # weights arrive host-relaid partition-major ([128, dm*128+k]) so each
    # projection loads in 4 big DMAs instead of 16 (the per-DMA trigger
    # cost on the issuing engine queue, ~0.65us, dominated the prologue
    # with 72 chunked loads).
    wqT = nc.dram_tensor("wqT", [128, NDM * RLOC], BF16, kind="ExternalInput")
    wkT = nc.dram_tensor("wkT", [128, NDM * RLOC], BF16, kind="ExternalInput")
    wvT = nc.dram_tensor("wvT", [128, NDM * RLOC], BF16, kind="ExternalInput")
    woTs = nc.dram_tensor("woTs", [128, 2048], BF16, kind="ExternalInput")
    bq = nc.dram_tensor("bq", [RLOC, 1], F32, kind="ExternalInput")
    bk = nc.dram_tensor("bk", [RLOC, 1], F32, kind="ExternalInput")
    bv = nc.dram_tensor("bv", [RLOC, 1], F32, kind="ExternalInput")
    bo2 = nc.dram_tensor("bo2", [128, 4], F32, kind="ExternalInput")
    outT = nc.dram_tensor("outT", [512, S], F32, kind="ExternalOutput")

    # Per-q-tile collective bounce buffers (chunked AllGather overlaps the
    # epilogue with attention of later q tiles). bf16 halves the wire.
    cc_in = [
        nc.dram_tensor(f"cc_in{i}", [RLOC, TQ], BF16, kind="Internal")
        for i in range(NQT)
    ]
    # NOTE: Shared-output collectives need >4-core groups; Local output is
    # the supported path for 4-core batch groups (extra HBM copy, fine).
    cc_out = [
        nc.dram_tensor(f"cc_out{i}", [R, TQ], BF16, kind="Internal")
        for i in range(NQT)
    ]
    replica_groups = [[0, 1, 2, 3], [4, 5, 6, 7]]

    with tile.TileContext(nc) as tc:
        with (
            tc.tile_pool(name="const", bufs=1) as const,
            tc.tile_pool(name="wpool", bufs=1) as wpool,
            tc.tile_pool(name="xpool", bufs=18) as xpool,
            tc.tile_pool(name="qkv", bufs=1) as qkv,
            tc.tile_pool(name="attnp", bufs=26) as attnp,
            tc.tile_pool(name="denp", bufs=2) as denp,
            tc.tile_pool(name="otp", bufs=2) as otp,
            tc.tile_pool(name="agp", bufs=8) as agp,
            tc.tile_pool(name="outp", bufs=2) as outp,
        ):
            # ---- constants / weights -------------------------------------
            # chunked weight loads: one DMA per 128x128 chunk so each matmul
            # waits on a single DMA-queue semaphore (a single sprayed DMA
            # fans across queues and overflows the ISA wait slots). All on
            # the scalar HWDGE queue: the sync queue carries the x stream
            # and must not serialize behind 68 weight descriptors.
            wq_sb = wpool.tile([128, NDM, RLOC], BF16)
            wk_sb = wpool.tile([128, NDM, RLOC], BF16)
            wv_sb = wpool.tile([128, NDM, RLOC], BF16)
            for c in range(4):
                cs = slice(4 * c, 4 * (c + 1))
                fs = slice(512 * c, 512 * (c + 1))
                nc.scalar.dma_start(wq_sb[:, cs, :], wqT[:, fs])
                nc.scalar.dma_start(wk_sb[:, cs, :], wkT[:, fs])
                nc.scalar.dma_start(wv_sb[:, cs, :], wvT[:, fs])
            wo_sb = wpool.tile([128, 4, 4, 128], BF16)
            for rc in range(4):
                nc.scalar.dma_start(
                    wo_sb[:, rc, :, :], woTs[:, 512 * rc : 512 * (rc + 1)]
                )
            bq_sb = const.tile([RLOC, 1], F32)
            bk_sb = const.tile([RLOC, 1], F32)
            bv_sb = const.tile([RLOC, 1], F32)
            bo_sb = const.tile([128, 4], F32)
            nc.scalar.dma_start(bq_sb, bq[:])
            nc.scalar.dma_start(bk_sb, bk[:])
            nc.scalar.dma_start(bv_sb, bv[:])
            nc.scalar.dma_start(bo_sb, bo2[:])

            ones_bf = const.tile([128, 32], BF16)
            nc.vector.memset(ones_bf, 1.0)
            ident = const.tile([128, 128], BF16)
            make_identity(nc, ident[:])

            # Warm the ACT exp table set during the prologue so the ~2.7us
            # PSEUDO_LOAD doesn't land in front of the first real exp.
            warm_in = const.tile([128, 1], F32)
            warm_out = const.tile([128, 1], F32)
            nc.vector.memset(warm_in, 0.0)
            nc.scalar.activation(
                warm_out[:], warm_in[:], mybir.ActivationFunctionType.Exp
            )

            # ---- pools ---------------------------------------------------
            # Phase 1 runs QKV projection interleaved with attention(0):
            # psum = proj accumulators (3) + score pool A (3) + AV (1) +
            # denominator (1) = 8 banks. Phase 2 (tiles 1-3) closes the
            # proj pool and opens score pool B in its place so scores
            # ping-pong A/B and the exp of one group overlaps the matmuls
            # of the next. out_proj runs entirely in the tail, in a 4-deep
            # pool opened after the attention pools close.
            qt = qkv.tile([RLOC, S], BF16)
            kt = qkv.tile([RLOC, S], BF16)
            vt_bf = qkv.tile([RLOC, S], BF16)
            v_bf = qkv.tile([128, NKT, 128], BF16)
            # pool release is stack-ordered: ps_proj and the 1-bank phase-1
            # score pool open last so they can close first when phase 2
            # swaps them for the 3-bank score pool B + denominator bank.
            # Phase-1 psum: proj 3 + scoreA 3 + scoreB1 1 + AV 1 = 8; the
            # softmax denominator matmuls for tile 0 are deferred to phase
            # 2 (the exp'd attention tiles stay resident in SBUF).
            ps_scA_ctx = tc.tile_pool(name="ps_scA", bufs=1, space="PSUM")
            ps_av_ctx = tc.tile_pool(name="ps_av", bufs=1, space="PSUM")
            ps_scB1_ctx = tc.tile_pool(name="ps_scB1", bufs=1, space="PSUM")
            ps_proj_ctx = tc.tile_pool(name="ps_proj", bufs=3, space="PSUM")
            ps_scA = ps_scA_ctx.__enter__()
            ps_av = ps_av_ctx.__enter__()
            ps_scB1 = ps_scB1_ctx.__enter__()
            ps_proj = ps_proj_ctx.__enter__()
            ps_den = [None]

            n_slots = NKT * NH  # 64 score tiles per q tile: slot = 4*kc + h

            ag_tiles = {}
            ps_out = [None]

            def emit_ag_loads(q):
                # On the scalar queue (idle once the exp stream ends),
                # pinned past the model makespan. They must NOT share a
                # queue with either the cc_in writes or the AG triggers: a
                # slow AllGather q would then stall the queue at this load
                # and delay AllGather q+1's trigger — measured as a
                # 100us+ cross-group cascade.
                ag_t = []
                for rc in range(GROUP):
                    t_ = agp.tile([128, TQ], BF16, tag="ag", name="ag")
                    nc.scalar.dma_start(t_, cc_out[q][128 * rc : 128 * (rc + 1), :])
                    ag_t.append(t_)
                ag_tiles[q] = ag_t

            def emit_out_proj(q):
                qsl = slice(TQ * q, TQ * (q + 1))
                ag_t = ag_tiles.pop(q)
                for dmt in range(4):
                    pso2 = ps_out[0].tile([128, TQ], F32, tag="op", name="op")
                    for rc in range(GROUP):
                        nc.tensor.matmul(
                            pso2[:],
                            wo_sb[:, rc, dmt, :],
                            ag_t[rc][:],
                            start=(rc == 0), stop=(rc == GROUP - 1),
                        )
                    ob = outp.tile([128, TQ], F32, tag="ob", name="ob")
                    nc.vector.tensor_scalar_add(ob[:], pso2[:], bo_sb[:, dmt : dmt + 1])
                    nc.sync.dma_start(outT[128 * dmt : 128 * (dmt + 1), qsl], ob[:])

            # ---- attention emission helpers ------------------------------
            def make_attn_state(q):
                pso = ps_av.tile([128, TQ], F32, tag="av", name="av")
                return {"q": q, "pso": pso, "psd": None, "slot_ap": {},
                        "g0": 0, "gi": 0, "next_kc": 0, "den_kc": 0}

            def emit_av(st_, kc):
                st = kc == 0
                sp = kc == NKT - 1
                for h in range(NH):
                    nc.tensor.matmul(
                        st_["pso"][32 * h : 32 * (h + 1), :],
                        v_bf[:, kc, 32 * h : 32 * (h + 1)],
                        st_["slot_ap"][NH * kc + h],
                        start=st, stop=sp,
                        tile_position=(0, 32 * h),
                    )

            def emit_den(st_, kc):
                if st_["psd"] is None:
                    st_["psd"] = ps_den[0].tile([128, TQ], F32, tag="den", name="den")
                st = kc == 0
                sp = kc == NKT - 1
                for h in range(NH):
                    nc.tensor.matmul(
                        st_["psd"][32 * h : 32 * (h + 1), :],
                        ones_bf[:, :],
                        st_["slot_ap"][NH * kc + h],
                        start=st, stop=sp,
                        tile_position=(0, 32 * h),
                    )

            def emit_attn_groups(st_, upto_slot, pools, with_den=True):
                # scores (bf16 in, fp32 psum) in up-to-3-slot groups + exp
                # (ACT). AV + denominator matmuls are emitted lagged one
                # pipeline stage behind the exp that produced their inputs:
                # an avden matmul whose exp is still in flight would sit at
                # the head of the in-order PE queue and block the next
                # score group, locking the pipeline to ACT round trips.
                q = st_["q"]
                qsl = slice(TQ * q, TQ * (q + 1))
                while st_["g0"] < upto_slot:
                    g0 = st_["g0"]
                    pool, cap = pools[st_["gi"] % len(pools)]
                    n = min(cap, upto_slot - g0)
                    pss = pool.tile([128, cap * TQ], F32, tag="sc", name="sc")
                    att = attnp.tile([128, cap * TQ], BF16,
                                     tag=f"at{cap}", name="at")
                    for s in range(n):
                        kc, h = divmod(g0 + s, NH)
                        nc.tensor.matmul(
                            pss[:, TQ * s : TQ * (s + 1)],
                            kt[32 * h : 32 * (h + 1), 128 * kc : 128 * (kc + 1)],
                            qt[32 * h : 32 * (h + 1), qsl],
                            start=True, stop=True,
                            tile_position=(32 * h, 0),
                        )
                    nc.scalar.activation(
                        att[:, : n * TQ], pss[:, : n * TQ],
                        mybir.ActivationFunctionType.Exp,
                    )
                    for s in range(n):
                        st_["slot_ap"][g0 + s] = att[:, TQ * s : TQ * (s + 1)]
                    st_["g0"] = g0 + n
                    st_["gi"] += 1
                    while (st_["next_kc"] + 1) * NH + 5 <= st_["g0"]:
                        emit_av(st_, st_["next_kc"])
                        if with_den:
                            emit_den(st_, st_["den_kc"])
                            st_["den_kc"] += 1
                        st_["next_kc"] += 1

            def finish_attention(st_):
                q = st_["q"]
                while st_["next_kc"] < NKT:
                    emit_av(st_, st_["next_kc"])
                    st_["next_kc"] += 1
                while st_["den_kc"] < NKT:
                    emit_den(st_, st_["den_kc"])
                    st_["den_kc"] += 1
                # out = AV / denom: the ones-matmul already broadcast each
                # head's denominator across its 32 rows.
                rb = denp.tile([128, TQ], F32, tag="rb", name="rb")
                nc.vector.reciprocal(rb[:], st_["psd"][:])
                ot = otp.tile([128, TQ], BF16, tag="ot", name="ot")
                nc.vector.tensor_mul(ot[:], st_["pso"][:], rb[:])
                nc.sync.dma_start(cc_in[q][:], ot[:])
                # gather the 4 cores' head-slices of this q tile
                nc.gpsimd.collective_compute(
                    "AllGather",
                    mybir.AluOpType.bypass,
                    replica_groups=replica_groups,
                    ins=[cc_in[q][:]],
                    outs=[cc_out[q][:]],
                )
                with tc.tile_wait_until(1.0):
                    emit_ag_loads(q)

            # ---- phase 1: QKV projection + attention(0) ------------------
            # Per token tile t: project q/k/v (all-bf16 operands, fp32 PSUM
            # accumulation; x tiles span two token tiles for 2KB DMA
            # lines), transpose the fresh V chunks, then emit attention(0)
            # score groups for the k-chunks this tile just produced. The
            # scheduler backfills the next tile's projection matmuls into
            # the PE idle slots of the exp round trips, so attention(0)'s
            # exp stream runs inside the projection span instead of after
            # it.
            st0 = make_attn_state(0)
            x_pair = {}
            for t in range(NQT):
                tsl = slice(TQ * t, TQ * (t + 1))
                accq = ps_proj.tile([128, TQ], F32, tag="proj", name="proj")
                acck = ps_proj.tile([128, TQ], F32, tag="proj", name="proj")
                accv = ps_proj.tile([128, TQ], F32, tag="proj", name="proj")
                for dm in range(NDM):
                    if t % 2 == 0:
                        xt_t = xpool.tile([128, 2 * TQ], BF16, tag="xt", name="xt")
                        psl = slice(TQ * t, TQ * (t + 2))
                        # split the x stream across the sync HWDGE queue
                        # and the gpsimd SWDGE queue.
                        if dm % 2 == 0:
                            nc.sync.dma_start(xt_t, xT[128 * dm : 128 * (dm + 1), psl])
                        else:
                            nc.gpsimd.dma_start(xt_t, xT[128 * dm : 128 * (dm + 1), psl])
                        x_pair[dm] = xt_t
                    xr = x_pair[dm][:, TQ * (t % 2) : TQ * (t % 2 + 1)]
                    for acc_, w_sb in ((accq, wq_sb), (acck, wk_sb), (accv, wv_sb)):
                        nc.tensor.matmul(
                            acc_[:], w_sb[:, dm, :], xr,
                            start=(dm == 0), stop=(dm == NDM - 1),
                        )
                nc.vector.tensor_scalar_add(qt[:, tsl], accq[:], bq_sb[:])
                nc.vector.tensor_scalar_add(kt[:, tsl], acck[:], bk_sb[:])
                nc.vector.tensor_scalar_add(vt_bf[:, tsl], accv[:], bv_sb[:])
                # V^T -> V for this tile's 4 k-chunks (natural [k,d] bf16)
                for c in range(4 * t, 4 * (t + 1)):
                    pst = ps_proj.tile([128, 128], BF16, tag="proj", name="proj")
                    nc.tensor.transpose(
                        pst[:], vt_bf[:, 128 * c : 128 * (c + 1)], ident[:]
                    )
                    nc.vector.tensor_copy(v_bf[:, c, :], pst[:])
                emit_attn_groups(
                    st0, 16 * (t + 1), [(ps_scA, 3), (ps_scB1, 1)],
                    with_den=False,
                )

            ps_proj_ctx.__exit__(None, None, None)
            ps_scB1_ctx.__exit__(None, None, None)
            ps_scB_ctx = tc.tile_pool(name="ps_scB", bufs=1, space="PSUM")
            ps_scB = ps_scB_ctx.__enter__()
            ps_den_ctx = tc.tile_pool(name="ps_den", bufs=1, space="PSUM")
            ps_den[0] = ps_den_ctx.__enter__()

            # tile 0's deferred denominator matmuls + epilogue, then
            # attention tiles 1-3; the scheduler interleaves the catch-up
            # den matmuls with tile 1's score stream.
            finish_attention(st0)

            # ---- phase 2: attention tiles 1-3 ----------------------------
            for q in range(1, NQT):
                st = make_attn_state(q)
                emit_attn_groups(st, n_slots, [(ps_scA, 3), (ps_scB, 3)])
                finish_attention(st)

            ps_den_ctx.__exit__(None, None, None)
            ps_scB_ctx.__exit__(None, None, None)
            ps_av_ctx.__exit__(None, None, None)
            ps_scA_ctx.__exit__(None, None, None)

            # tail: all out_proj, 4-deep so matmul groups pipeline past the
            # DVE bias-drains; op(0..2)'s AllGathers are long done, so this
            # work fills the AllGather(3) completion window.
            ps_out_ctx = tc.tile_pool(name="ps_out", bufs=4, space="PSUM")
            ps_out[0] = ps_out_ctx.__enter__()
            with tc.tile_wait_until(1.0):
                for q in range(NQT):
                    emit_out_proj(q)
            ps_out_ctx.__exit__(None, None, None)

    nc.finalize()
    return nc


def _prepare_inputs(x, Wq, bq, Wk, bk, Wv, bv, Wo, bo):
    import ml_dtypes

    bf16 = ml_dtypes.bfloat16

    def pmajor(wT):
        # [2048, 128] -> [128, 16*128]: row p holds chunk-major weights so
        # the kernel can load 4 d_model chunks per DMA with 128 partitions.
        return np.ascontiguousarray(
            wT.reshape(NDM, 128, RLOC).transpose(1, 0, 2).reshape(128, NDM * RLOC)
        )

    scale = 1.0 / math.sqrt(D_K)
    x = np.asarray(x, np.float32)
    in_maps = []
    for c in range(N_CORES):
        b, j = divmod(c, GROUP)
        rsl = slice(RLOC * j, RLOC * (j + 1))
        dsl = slice(512 * j, 512 * (j + 1))
        woT = np.asarray(Wo)[dsl].T.astype(bf16)  # [512 r, 512 dm-slice]
        wo_pm = np.ascontiguousarray(
            woT.reshape(4, 128, 4, 128).transpose(1, 0, 2, 3).reshape(128, 2048)
        )
        in_maps.append(
            {
                "xT": np.ascontiguousarray(x[b].T.astype(bf16)),
                "wqT": pmajor((np.asarray(Wq)[rsl] * scale).T.astype(bf16)),
                "wkT": pmajor(np.asarray(Wk)[rsl].T.astype(bf16)),
                "wvT": pmajor(np.asarray(Wv)[rsl].T.astype(bf16)),
                "woTs": wo_pm,
                "bq": (np.asarray(bq)[rsl] * scale).astype(np.float32).reshape(RLOC, 1),
                "bk": np.asarray(bk)[rsl].astype(np.float32).reshape(RLOC, 1),
                "bv": np.asarray(bv)[rsl].astype(np.float32).reshape(RLOC, 1),
                "bo2": np.ascontiguousarray(
                    np.asarray(bo)[dsl].astype(np.float32).reshape(4, 128).T
                ),
            }
        )
    return in_maps


def kernel(x, Wq, bq, Wk, bk, Wv, bv, Wo, bo, mask=None):
    global LAST_RESULT
    from concourse.bass_utils import run_bass_kernel_spmd

    if "nc" not in _CACHE:
        _CACHE["nc"] = _build()
    nc = _CACHE["nc"]

    in_maps = _prepare_inputs(x, Wq, bq, Wk, bk, Wv, bv, Wo, bo)
    res = run_bass_kernel_spmd(
        nc, in_maps, core_ids=list(range(N_CORES)), trace=TRACE
    )
    LAST_RESULT = res
    out = np.empty((B, S, D_MODEL), np.float32)
    for c in range(N_CORES):
        b, j = divmod(c, GROUP)
        out[b, :, 512 * j : 512 * (j + 1)] = res.results[c]["outT"].T
    return out
